# revision 11
# baseline (speedup 1.0000x reference)
"""Trainium2 Bass kernel for the scalar-gain Kalman filter.

Math: the reference recurrence x_k = x_{k-1} + K_k (z_k - x_{k-1}) has
data-independent scalar gains K_k (they depend only on log_Q/log_R), so
the whole filter is a linear map along the time axis:

    x[n, k] = sum_j L[k, j] * z[n, j],   L[k, j] = K_j * prod_{i=j+1..k} (1 - K_i)

with K_0 := 1.  L is lower-triangular 512x512, computed on the host from
the two scalar params.  Because |1-K_i| converges to ~0.382, L[k, j]
decays geometrically in (k-j); entries with k-j >= 128 are < 1e-53, so
restricting L to a 2-block band (current + previous 128-wide time chunk)
is exact at f32 precision.

Device kernel (per core, pure data parallel over rows):
  - shard: z [64,1024,512] -> 8 shards of [8192, 512] rows (batch split)
  - per row-tile [128, 512]: DMA in, 4x PE transpose (128x128, via
    identity) -> PSUM -> ACT copy to SBUF, then 7 banded matmuls
    accumulating out[:, kc] += zT_chunk.T @ LT_block in PSUM, DVE copy
    PSUM->SBUF, DMA out.
"""

import numpy as np

import concourse.bass as bass
import concourse.mybir as mybir
from concourse import bacc
from concourse import bass_utils
from concourse.tile import TileContext
from concourse.masks import make_identity

B, C, W = 64, 1024, 512
NCORES = 8
ROWS = B * C // NCORES  # 8192 rows per core
P = 128                 # partitions / row-tile height
NT = ROWS // P          # 64 row-tiles per core
CH = 128                # time chunk
NCH = W // CH           # 4 chunks
NBLK = 2 * NCH - 1      # 7 banded LT blocks

_cache = {}


def _build_nc():
    nc = bacc.Bacc(
        "TRN2",
        target_bir_lowering=False,
        debug=False,
        enable_asserts=False,
        num_devices=NCORES,
    )
    z = nc.dram_tensor("z", [ROWS, W], mybir.dt.float32, kind="ExternalInput").ap()
    lt = nc.dram_tensor("lt", [P, NBLK * CH], mybir.dt.float32, kind="ExternalInput").ap()
    out = nc.dram_tensor("out", [ROWS, W], mybir.dt.float32, kind="ExternalOutput").ap()

    with TileContext(nc) as tc:
        with (
            tc.tile_pool(name="const", bufs=1) as constp,
            tc.tile_pool(name="zin", bufs=8) as zinp,
            tc.tile_pool(name="zt", bufs=34) as ztp,
            tc.tile_pool(name="res", bufs=4) as resp,
            tc.tile_pool(name="trps", bufs=5, space="PSUM") as trpsp,
            tc.tile_pool(name="outps", bufs=3, space="PSUM") as outpsp,
        ):
            ltt = constp.tile([P, NBLK * CH], mybir.dt.float32)
            nc.sync.dma_start(ltt[:], lt)
            ident = constp.tile([P, P], mybir.dt.float32)
            make_identity(nc, ident[:])

            # Process in groups: a burst of transposes, then a burst of
            # matmuls, so the PE sees dense matmul stretches (HAM warmth).
            G = 32
            for g in range(NT // G):
                zts = []
                for ti in range(G):
                    t = g * G + ti
                    zin = zinp.tile([P, W], mybir.dt.float32)
                    nc.sync.dma_start(zin[:], z[t * P : (t + 1) * P, :])
                    # transpose the four 128x128 chunks: zt[:, q] = (z chunk q)^T
                    zt = ztp.tile([P, W], mybir.dt.float32)
                    for q in range(NCH):
                        trps = trpsp.tile([P, CH], mybir.dt.float32)
                        nc.tensor.transpose(
                            trps[:], zin[:, q * CH : (q + 1) * CH], ident[:]
                        )
                        if q % 2 == 0:
                            nc.scalar.copy(zt[:, q * CH : (q + 1) * CH], trps[:])
                        else:
                            nc.vector.tensor_copy(
                                zt[:, q * CH : (q + 1) * CH], trps[:]
                            )
                    zts.append(zt)

                for ti in range(G):
                    t = g * G + ti
                    zt = zts[ti]
                    # merged banded matmuls: matmul q covers (diag_q | prev_{q+1})
                    # = out columns [128q, 128q+256), one stationary load each.
                    # start=True on q=0 clears has_written for the whole bank;
                    # later matmuls overwrite fresh columns, accumulate covered.
                    ops = outpsp.tile([P, W], mybir.dt.float32)
                    for q in range(NCH):
                        ncols = 2 * CH if q < NCH - 1 else CH
                        nc.tensor.matmul(
                            ops[:, q * CH : q * CH + ncols],
                            zt[:, q * CH : (q + 1) * CH],
                            ltt[:, 2 * q * CH : 2 * q * CH + ncols],
                            start=(q == 0),
                            stop=(q == NCH - 1),
                            skip_group_check=True,
                        )

                    res = resp.tile([P, W], mybir.dt.float32)
                    if ti % 2 == 0:
                        nc.vector.tensor_copy(res[:], ops[:])
                    else:
                        nc.scalar.copy(res[:], ops[:])
                    nc.gpsimd.dma_start(out[t * P : (t + 1) * P, :], res[:])
    nc.compile()
    return nc


def _gains(log_Q, log_R):
    """Replicate the reference f32 scalar scan for the Kalman gains."""
    f32 = np.float32
    Q = f32(np.exp(f32(log_Q)))
    R = f32(np.exp(f32(log_R)))
    Pv = f32(Q + R)
    Ks = np.empty(W, np.float64)
    Ks[0] = 1.0  # x_0 = z_0
    for k in range(1, W):
        P_pred = f32(Pv + Q)
        K = f32(P_pred / f32(P_pred + R))
        Pv = f32(f32(1.0 - K) * P_pred)
        Ks[k] = K
    return Ks


def _lt_pack(log_Q, log_R):
    """Banded blocks of L^T, packed [128, 7*128] f32.

    Block b holds LT_block = L[kc_range, jc_range]^T with partition = j
    (contraction dim), free = k.  Order: (c0,diag), (c1,prev), (c1,diag),
    (c2,prev), (c2,diag), (c3,prev), (c3,diag).
    """
    Ks = _gains(log_Q, log_R)
    a = 1.0 - Ks
    a[0] = 1.0
    cp = np.cumprod(a)  # cp[k] = prod_{i<=k} a_i  (a_0 = 1)
    # L[k, j] = Ks[j] * cp[k] / cp[j]  for j <= k
    k_idx = np.arange(W)
    Lf = Ks[None, :] * (cp[:, None] / cp[None, :])
    Lf = np.where(k_idx[None, :] <= k_idx[:, None], Lf, 0.0)

    # Layout: for q in 0..3 -> [diag_q | prev_{q+1}] adjacent, so one
    # matmul with stationary zt[q] covers out columns [128q, 128q+256).
    blocks = []
    for q in range(NCH):
        js = slice(q * CH, (q + 1) * CH)
        blocks.append(Lf[js, js].T)  # diag_q : k-chunk q, j-chunk q
        if q < NCH - 1:
            ks = slice((q + 1) * CH, (q + 2) * CH)
            blocks.append(Lf[ks, js].T)  # prev_{q+1} : k-chunk q+1, j-chunk q
    return np.ascontiguousarray(np.concatenate(blocks, axis=1).astype(np.float32))


def _get_nc():
    nc = _cache.get("nc")
    if nc is None:
        nc = _build_nc()
        _cache["nc"] = nc
    return nc


def run_sharded(z, log_Q, log_R, **spmd_kwargs):
    """Run the SPMD kernel; returns (full_output, BassKernelResults)."""
    nc = _get_nc()
    ltp = _lt_pack(np.asarray(log_Q).reshape(-1)[0], np.asarray(log_R).reshape(-1)[0])
    zsh = np.ascontiguousarray(np.asarray(z, np.float32).reshape(NCORES, ROWS, W))
    in_maps = [{"z": zsh[i], "lt": ltp} for i in range(NCORES)]
    res = bass_utils.run_bass_kernel_spmd(
        nc, in_maps, core_ids=list(range(NCORES)), **spmd_kwargs
    )
    full = np.concatenate([r["out"] for r in res.results], axis=0).reshape(B, C, W)
    return full, res


def kernel(z, log_Q, log_R):
    full, _ = run_sharded(z, log_Q, log_R)
    return full


# revision 12
# speedup vs baseline: 1.0303x; 1.0303x over previous
"""Trainium2 Bass kernel for the scalar-gain Kalman filter.

Math: the reference recurrence x_k = x_{k-1} + K_k (z_k - x_{k-1}) has
data-independent scalar gains K_k (they depend only on log_Q/log_R), so
the whole filter is a linear map along the time axis:

    x[n, k] = sum_j L[k, j] * z[n, j],   L[k, j] = K_j * prod_{i=j+1..k} (1 - K_i)

with K_0 := 1.  L is lower-triangular 512x512, computed on the host from
the two scalar params.  Because |1-K_i| converges to ~0.382, L[k, j]
decays geometrically in (k-j); entries with k-j >= 128 are < 1e-53, so
restricting L to a 2-block band (current + previous 128-wide time chunk)
is exact at f32 precision.

Device kernel (per core, pure data parallel over rows):
  - shard: z [64,1024,512] -> 8 shards of [8192, 512] rows (batch split)
  - per row-tile [128, 512]: DMA in, 4x PE transpose (128x128, via
    identity) -> PSUM -> ACT copy to SBUF, then 7 banded matmuls
    accumulating out[:, kc] += zT_chunk.T @ LT_block in PSUM, DVE copy
    PSUM->SBUF, DMA out.
"""

import numpy as np

import concourse.bass as bass
import concourse.mybir as mybir
from concourse import bacc
from concourse import bass_utils
from concourse.tile import TileContext
from concourse.masks import make_identity

B, C, W = 64, 1024, 512
NCORES = 8
ROWS = B * C // NCORES  # 8192 rows per core
P = 128                 # partitions / row-tile height
NT = ROWS // P          # 64 row-tiles per core
CH = 128                # time chunk
NCH = W // CH           # 4 chunks
NBLK = 2 * NCH - 1      # 7 banded LT blocks

_cache = {}


def _build_nc():
    nc = bacc.Bacc(
        "TRN2",
        target_bir_lowering=False,
        debug=False,
        enable_asserts=False,
        num_devices=NCORES,
    )
    z = nc.dram_tensor("z", [ROWS, W], mybir.dt.float32, kind="ExternalInput").ap()
    lt = nc.dram_tensor("lt", [P, NBLK * CH], mybir.dt.float32, kind="ExternalInput").ap()
    out = nc.dram_tensor("out", [ROWS, W], mybir.dt.float32, kind="ExternalOutput").ap()

    with TileContext(nc) as tc:
        with (
            tc.tile_pool(name="const", bufs=1) as constp,
            tc.tile_pool(name="zin", bufs=8) as zinp,
            tc.tile_pool(name="zt", bufs=34) as ztp,
            tc.tile_pool(name="res", bufs=4) as resp,
            tc.tile_pool(name="trps", bufs=5, space="PSUM") as trpsp,
            tc.tile_pool(name="outps", bufs=3, space="PSUM") as outpsp,
        ):
            ltt = constp.tile([P, NBLK * CH], mybir.dt.float32)
            nc.sync.dma_start(ltt[:], lt)
            ident = constp.tile([P, P], mybir.dt.float32)
            make_identity(nc, ident[:])

            # Process in groups: a burst of transposes, then a burst of
            # matmuls, so the PE sees dense matmul stretches (HAM warmth).
            G = 32
            for g in range(NT // G):
                zts = []
                for ti in range(G):
                    t = g * G + ti
                    zin = zinp.tile([P, W], mybir.dt.float32)
                    nc.sync.dma_start(zin[:], z[t * P : (t + 1) * P, :])
                    # transpose the four 128x128 chunks: zt[:, q] = (z chunk q)^T
                    zt = ztp.tile([P, W], mybir.dt.float32)
                    for q in range(NCH):
                        trps = trpsp.tile([P, CH], mybir.dt.float32)
                        nc.tensor.transpose(
                            trps[:], zin[:, q * CH : (q + 1) * CH], ident[:]
                        )
                        if q % 2 == 0:
                            nc.scalar.copy(zt[:, q * CH : (q + 1) * CH], trps[:])
                        else:
                            nc.vector.tensor_copy(
                                zt[:, q * CH : (q + 1) * CH], trps[:]
                            )
                    zts.append(zt)

                for ti in range(G):
                    t = g * G + ti
                    zt = zts[ti]
                    # merged banded matmuls: matmul q covers (diag_q | prev_{q+1})
                    # = out columns [128q, 128q+256), one stationary load each.
                    # start=True on q=0 clears has_written for the whole bank;
                    # later matmuls overwrite fresh columns, accumulate covered.
                    ops = outpsp.tile([P, W], mybir.dt.float32)
                    for q in range(NCH):
                        ncols = 2 * CH if q < NCH - 1 else CH
                        nc.tensor.matmul(
                            ops[:, q * CH : q * CH + ncols],
                            zt[:, q * CH : (q + 1) * CH],
                            ltt[:, 2 * q * CH : 2 * q * CH + ncols],
                            start=(q == 0),
                            stop=(q == NCH - 1),
                            skip_group_check=True,
                        )

                    res = resp.tile([P, W], mybir.dt.float32)
                    if ti % 2 == 0:
                        nc.vector.tensor_copy(res[:], ops[:])
                    else:
                        nc.scalar.copy(res[:], ops[:])
                    nc.sync.dma_start(out[t * P : (t + 1) * P, :], res[:])
    nc.compile()
    return nc


def _gains(log_Q, log_R):
    """Replicate the reference f32 scalar scan for the Kalman gains."""
    f32 = np.float32
    Q = f32(np.exp(f32(log_Q)))
    R = f32(np.exp(f32(log_R)))
    Pv = f32(Q + R)
    Ks = np.empty(W, np.float64)
    Ks[0] = 1.0  # x_0 = z_0
    for k in range(1, W):
        P_pred = f32(Pv + Q)
        K = f32(P_pred / f32(P_pred + R))
        Pv = f32(f32(1.0 - K) * P_pred)
        Ks[k] = K
    return Ks


def _lt_pack(log_Q, log_R):
    """Banded blocks of L^T, packed [128, 7*128] f32.

    Block b holds LT_block = L[kc_range, jc_range]^T with partition = j
    (contraction dim), free = k.  Order: (c0,diag), (c1,prev), (c1,diag),
    (c2,prev), (c2,diag), (c3,prev), (c3,diag).
    """
    Ks = _gains(log_Q, log_R)
    a = 1.0 - Ks
    a[0] = 1.0
    cp = np.cumprod(a)  # cp[k] = prod_{i<=k} a_i  (a_0 = 1)
    # L[k, j] = Ks[j] * cp[k] / cp[j]  for j <= k
    k_idx = np.arange(W)
    Lf = Ks[None, :] * (cp[:, None] / cp[None, :])
    Lf = np.where(k_idx[None, :] <= k_idx[:, None], Lf, 0.0)

    # Layout: for q in 0..3 -> [diag_q | prev_{q+1}] adjacent, so one
    # matmul with stationary zt[q] covers out columns [128q, 128q+256).
    blocks = []
    for q in range(NCH):
        js = slice(q * CH, (q + 1) * CH)
        blocks.append(Lf[js, js].T)  # diag_q : k-chunk q, j-chunk q
        if q < NCH - 1:
            ks = slice((q + 1) * CH, (q + 2) * CH)
            blocks.append(Lf[ks, js].T)  # prev_{q+1} : k-chunk q+1, j-chunk q
    return np.ascontiguousarray(np.concatenate(blocks, axis=1).astype(np.float32))


def _get_nc():
    nc = _cache.get("nc")
    if nc is None:
        nc = _build_nc()
        _cache["nc"] = nc
    return nc


def run_sharded(z, log_Q, log_R, **spmd_kwargs):
    """Run the SPMD kernel; returns (full_output, BassKernelResults)."""
    nc = _get_nc()
    ltp = _lt_pack(np.asarray(log_Q).reshape(-1)[0], np.asarray(log_R).reshape(-1)[0])
    zsh = np.ascontiguousarray(np.asarray(z, np.float32).reshape(NCORES, ROWS, W))
    in_maps = [{"z": zsh[i], "lt": ltp} for i in range(NCORES)]
    res = bass_utils.run_bass_kernel_spmd(
        nc, in_maps, core_ids=list(range(NCORES)), **spmd_kwargs
    )
    full = np.concatenate([r["out"] for r in res.results], axis=0).reshape(B, C, W)
    return full, res


def kernel(z, log_Q, log_R):
    full, _ = run_sharded(z, log_Q, log_R)
    return full
